# revision 13
# baseline (speedup 1.0000x reference)
"""Trainium2 Bass kernel for a 3D-gaussian-splatting rasterizer.

Transposed ("pixels on partitions") design:
  host (numpy, O(N) work): quaternion -> cov3D -> EWA cov2D -> conic,
    projection, depth sort, per-half-cell (8x16 = 128 px) culling. The 128
    half-cells are LPT-balanced over 8 cores; each core gets one column
    strip: [sep][cell0 gaussians][sep][cell1 gaussians]... padded to TP.
  device (8 NeuronCores, SPMD), partitions = the 128 pixels of whichever
    cell a column belongs to, free dim = gaussian columns:
      P      = basisT^T @ coef                   (one [6,128] stationary)
      alpha  = exp(P)                            (ONE scalar-engine pass)
      om     = 1 - alpha                         (DVE tensor_scalar)
      ts     = scan: state = max(om*state, rst)  (DVE tensor_tensor_scan;
               rst=1 at separator columns resets the running transmittance
               product exactly to 1 between cells -- state is always <=1)
      tt_c   = transpose(ts chunk)               (PE transpose via identity)
      R     += tt_c^T @ d_chunk                  (accumulating rgb matmul)
    where d is the Abel-summation difference array: for cell block b,
      d[sep_b]  = first color (or bg if empty),  d[t] = c_{t+1} - c_t,
      d[last_b] = bg - c_last
    so  rgb_b = sum_t ts[p,t] * d[t, 3b:3b+3]  == front-to-back compositing
    including the residual-transmittance * bg term. No per-cell row caps,
    no chunk chaining, no slot gating, and only ONE exp pass per element.
  host: scatter the [128 px, 3] per-cell blocks into the [3,128,128] image.

Numerically validated (sim2.py): f32 coef/basis + fp16 scan/colors
gives rel err ~6e-3 vs the 2e-2 gate (bf16 coef fails: cancellation).
"""

import os
import numpy as np

N_CORES = 8
H = W = 128
NEG_BIG = -1.0e9
ZNEAR = 0.2
TANFOV = 0.5
FOCAL = W / (2.0 * TANFOV)   # 128.0
KB = 128
NCELL = 16                   # half-cells per core (128/8)
NC3 = 3 * NCELL

_compiled_cache = {}


# ----------------------------------------------------------------------------
# Host-side per-gaussian preprocessing (numpy, O(N))
# ----------------------------------------------------------------------------

def _preprocess(means3D, opacities, colors_precomp, scales, rotations, viewmatrix):
    q = rotations / np.linalg.norm(rotations, axis=-1, keepdims=True)
    r, x, y, z = q[:, 0], q[:, 1], q[:, 2], q[:, 3]
    R = np.stack([
        1 - 2 * (y * y + z * z), 2 * (x * y - r * z), 2 * (x * z + r * y),
        2 * (x * y + r * z), 1 - 2 * (x * x + z * z), 2 * (y * z - r * x),
        2 * (x * z - r * y), 2 * (y * z + r * x), 1 - 2 * (x * x + y * y),
    ], axis=-1).reshape(-1, 3, 3)
    M = R * scales[:, None, :]
    cov3D = np.einsum('nij,nkj->nik', M, M)

    Wm = viewmatrix[:3, :3]
    t = means3D @ Wm.T + viewmatrix[:3, 3]
    tz = t[:, 2]
    lim = 1.3 * TANFOV
    txz = np.clip(t[:, 0] / tz, -lim, lim) * tz
    tyz = np.clip(t[:, 1] / tz, -lim, lim) * tz
    zero = np.zeros_like(tz)
    fx = fy = FOCAL
    J = np.stack([
        np.stack([fx / tz, zero, -fx * txz / (tz * tz)], axis=-1),
        np.stack([zero, fy / tz, -fy * tyz / (tz * tz)], axis=-1),
    ], axis=1)
    T = np.einsum('nij,jk->nik', J, Wm)
    cov2D = np.einsum('nij,njk,nlk->nil', T, cov3D, T)
    a = cov2D[:, 0, 0] + 0.3
    b = cov2D[:, 0, 1]
    c = cov2D[:, 1, 1] + 0.3
    det = a * c - b * b
    det_safe = np.where(det > 0, det, 1.0)
    conA, conB, conC = c / det_safe, -b / det_safe, a / det_safe
    px = fx * t[:, 0] / tz + (W - 1) * 0.5
    py = fy * t[:, 1] / tz + (H - 1) * 0.5
    valid = (det > 0) & (tz > ZNEAR)
    opac = opacities[:, 0]

    # bounding half-widths of the {alpha >= 1/255} ellipse
    ell = np.log(np.maximum(opac * 255.0, 1.0 + 1e-7))
    rx = np.where(valid, np.sqrt(np.maximum(2 * ell * a, 0.0)), 0.0)
    ry = np.where(valid, np.sqrt(np.maximum(2 * ell * c, 0.0)), 0.0)

    order = np.argsort(tz, kind='stable')
    d = dict(conA=conA, conB=conB, conC=conC, px=px, py=py, opac=opac,
             cols=colors_precomp, valid=valid, rx=rx, ry=ry, ell=ell)
    return {k: v[order] for k, v in d.items()}


def _cull_rect(pre, xlo, ylo, w, h):
    """Indices (depth-ordered) of gaussians touching rect, ellipse-corner
    refined."""
    px, py, rx, ry = pre['px'], pre['py'], pre['rx'], pre['ry']
    xhi, yhi = xlo + w - 1, ylo + h - 1
    hit = pre['valid'] & (px + rx >= xlo) & (px - rx <= xhi) \
        & (py + ry >= ylo) & (py - ry <= yhi)
    cx = np.clip(px, xlo, xhi)
    cy = np.clip(py, ylo, yhi)
    dx = cx - px
    dy = cy - py
    beyond = (dx != 0) & (dy != 0)
    quad = pre['conA'] * dx * dx + 2 * pre['conB'] * dx * dy \
        + pre['conC'] * dy * dy
    hit &= ~beyond | (quad <= 2 * pre['ell'])
    return np.nonzero(hit)[0]


def _make_basisT():
    """[6, 128]: rows x^2,y^2,xy,x,y,1 over the 8x16 cell (p = yy*8+xx)."""
    b = np.zeros((6, 128), np.float32)
    for yy in range(16):
        for xx in range(8):
            p = yy * 8 + xx
            xr, yr = xx - 3.5, yy - 7.5
            b[:, p] = [xr * xr, yr * yr, xr * yr, xr, yr, 1.0]
    return b


def _build_core_arrays(pre, cells, TP, bg):
    """coef [6, TP] f32, d [TP, NC3] f32, rst [TP] f32 for one core."""
    coef = np.zeros((6, TP), np.float32)
    coef[5, :] = NEG_BIG
    d = np.zeros((TP, NC3), np.float32)
    rst = np.zeros(TP, np.float32)
    col = 0
    for bi, (ti, tj, sx, idx) in enumerate(cells):
        vx = tj * 16 + 8 * sx + 3.5
        vy = ti * 16 + 7.5
        rst[col] = 1.0
        n = len(idx)
        cols_g = pre['cols'][idx]
        d[col, 3 * bi:3 * bi + 3] = cols_g[0] if n else bg
        col += 1
        if n:
            A, Bc, C = pre['conA'][idx], pre['conB'][idx], pre['conC'][idx]
            pxr = pre['px'][idx] - vx
            pyr = pre['py'][idx] - vy
            sl = slice(col, col + n)
            coef[0, sl] = -0.5 * A
            coef[1, sl] = -0.5 * C
            coef[2, sl] = -Bc
            coef[3, sl] = A * pxr + Bc * pyr
            coef[4, sl] = C * pyr + Bc * pxr
            coef[5, sl] = -0.5 * (A * pxr * pxr + C * pyr * pyr) \
                - Bc * pxr * pyr + np.log(pre['opac'][idx])
            dd = np.empty((n, 3), np.float32)
            dd[:-1] = cols_g[1:] - cols_g[:-1]
            dd[-1] = bg - cols_g[-1]
            d[col:col + n, 3 * bi:3 * bi + 3] = dd
            col += n
    return coef, d, rst


# ----------------------------------------------------------------------------
# Device program
# ----------------------------------------------------------------------------

def _build_program(TP):
    from contextlib import ExitStack
    import concourse.bass as bass  # noqa: F401
    import concourse.tile as tile
    from concourse import mybir, bacc

    f32 = mybir.dt.float32
    f32r = mybir.dt.float32r
    fp16 = mybir.dt.float16
    AF = mybir.ActivationFunctionType
    ALU = mybir.AluOpType

    CH = TP // KB                      # transpose/rgb chunks
    # head-pipelined power/exp/om segments: small first segment so the
    # first scan chunk starts as early as possible
    if TP > 512:
        segs = [(0, 128), (128, 512), (512, TP)]
    else:
        segs = [(0, 128), (128, TP)] if TP > 128 else [(0, TP)]
    SEG0 = 128

    class _BaccOneActSet(bacc.Bacc):
        # Pin Exp to one table set so the scalar engine loads tables once.
        def insert_act_table_loads(self):
            from concourse.hw_specs import get_activation_tables
            from concourse.bacc import _bass_rust
            tables = []
            for name, fns in get_activation_tables(self.m.arch).items():
                if name != 'natural_log_exp_and_others':
                    fns = fns - {AF.Exp}
                tables.append((name, fns))
            _bass_rust.insert_act_table_loads(self, tables)

    nc = _BaccOneActSet(None)
    bc0_d = nc.declare_dram_parameter("bc0", [6, KB + SEG0], f32r,
                                      isOutput=False)
    cfr_d = None
    if TP > SEG0:
        cfr_d = nc.declare_dram_parameter("cfr", [6, TP - SEG0], f32r,
                                          isOutput=False)
    assert segs[0] == (0, SEG0)
    # ors = [ones(128) ++ reset-row(TP)]: the ones give a K=1 broadcast
    # matmul that replicates the reset row across all 128 partitions.
    ors_d = nc.declare_dram_parameter("ors", [1, KB + TP], f32r,
                                      isOutput=False)
    dxp_d = nc.declare_dram_parameter("dxp", [KB, CH * NC3], fp16,
                                      isOutput=False)
    idn_d = nc.declare_dram_parameter("idn", [KB, KB], fp16, isOutput=False)
    orgb_d = nc.declare_dram_parameter("orgb", [KB, NC3], fp16, isOutput=True)

    with ExitStack() as ctx:
        tc = ctx.enter_context(tile.TileContext(
            nc, linearize=bool(int(os.environ.get("GR_LINEARIZE", "0")))))
        const_pool = ctx.enter_context(tc.tile_pool(name="const", bufs=1))
        ps = ctx.enter_context(tc.tile_pool(name="psum", bufs=1, space="PSUM"))

        bc_sb = const_pool.tile([6, KB + TP], f32r)
        ors_sb = const_pool.tile([1, KB + TP], f32r)
        dxp_sb = const_pool.tile([KB, CH * NC3], fp16)
        idn_sb = const_pool.tile([KB, KB], fp16)
        alpha_sb = const_pool.tile([KB, TP], f32)
        om_sb = const_pool.tile([KB, TP], fp16)
        ts_sb = const_pool.tile([KB, TP], fp16)
        tt_sb = const_pool.tile([KB, TP], fp16)
        out_sb = const_pool.tile([KB, NC3], fp16)

        # input DMAs: everything on the sync HW-DGE queue except the big
        # dxp (gpsimd's own queue); the scalar queue stays free so its
        # activation-table loads finish before the first exp needs them.
        nc.sync.dma_start(bc_sb[:, 0:KB + SEG0], bc0_d[:])
        nc.sync.dma_start(ors_sb[:], ors_d[:])
        if cfr_d is not None:
            nc.sync.dma_start(bc_sb[:, KB + SEG0:], cfr_d[:])
        nc.sync.dma_start(idn_sb[:], idn_d[:])
        nc.gpsimd.dma_start(dxp_sb[:], dxp_d[:])

        basisT = bc_sb[:, 0:KB]
        # reset row broadcast to 128 partitions via K=1 matmuls
        r_ps = ps.tile([KB, TP], f32, tag="rps", bufs=1)
        rsegs = [(a, min(a + 512, TP)) for a in range(0, TP, 512)]

        def e_rones(i):
            a, b = rsegs[i]
            nc.tensor.matmul(r_ps[:, a:b], lhsT=ors_sb[:, 0:KB],
                             rhs=ors_sb[:, KB + a:KB + b],
                             start=True, stop=True)

        P = [None] * len(segs)

        def e_pow(si):
            a, b = segs[si]
            P[si] = ps.tile([KB, b - a], f32, tag=f"p{si}", bufs=1,
                            name=f"P{si}")
            nc.tensor.matmul(P[si][:], lhsT=basisT,
                             rhs=bc_sb[:, KB + a:KB + b],
                             start=True, stop=True)

        def e_act(si):
            a, b = segs[si]
            nc.scalar.activation(alpha_sb[:, a:b], P[si][:], AF.Exp)
            # om = 1 - alpha on the scalar engine too: no cross-engine hop
            # and the vector engine stays dedicated to the scan chain
            nc.scalar.activation(om_sb[:, a:b], alpha_sb[:, a:b],
                                 AF.Identity, bias=1.0, scale=-1.0)

        e_pow(0)
        e_rones(0)
        e_act(0)
        for si in range(1, len(segs)):
            e_pow(si)
            if si < len(rsegs):
                e_rones(si)
            e_act(si)
        for i in range(len(segs), len(rsegs)):
            e_rones(i)

        for c in range(CH):
            sl = slice(c * KB, (c + 1) * KB)
            nc.vector.tensor_tensor_scan(
                ts_sb[:, sl], om_sb[:, sl], r_ps[:, sl],
                initial=(1.0 if c == 0 else ts_sb[:, c * KB - 1:c * KB]),
                op0=ALU.mult, op1=ALU.max)

        R = ps.tile([KB, NC3], f32, tag="r", bufs=1)
        for c in range(CH):
            sl = slice(c * KB, (c + 1) * KB)
            TPc = ps.tile([KB, KB], fp16, tag="tp", bufs=2, name=f"TP{c}")
            nc.tensor.transpose(TPc[:], ts_sb[:, sl], idn_sb[:])
            # PSUM -> SBUF chunk copies alternate scalar/vector engines
            # (gpsimd cannot access PSUM)
            if c % 2 == 0:
                nc.scalar.copy(tt_sb[:, sl], TPc[:])
            else:
                nc.vector.tensor_copy(tt_sb[:, sl], TPc[:])
            nc.tensor.matmul(R[:], lhsT=tt_sb[:, sl],
                             rhs=dxp_sb[:, c * NC3:(c + 1) * NC3],
                             start=(c == 0), stop=(c == CH - 1))

        nc.vector.tensor_copy(out_sb[:], R[:])
        nc.sync.dma_start(orgb_d[:], out_sb[:])

    nc.compile()
    return nc


# ----------------------------------------------------------------------------
# Entry point
# ----------------------------------------------------------------------------

def kernel(means3D, means2D, opacities, colors_precomp, scales, rotations,
           bg, viewmatrix):
    import ml_dtypes
    fp16 = np.float16
    means3D = np.asarray(means3D, np.float32)
    opacities = np.asarray(opacities, np.float32)
    colors_precomp = np.asarray(colors_precomp, np.float32)
    scales = np.asarray(scales, np.float32)
    rotations = np.asarray(rotations, np.float32)
    bg = np.asarray(bg, np.float32)
    viewmatrix = np.asarray(viewmatrix, np.float32)

    pre = _preprocess(means3D, opacities, colors_precomp, scales, rotations,
                      viewmatrix)

    # per half-cell culled lists, LPT-balanced over cores
    cells = []
    for ti in range(8):
        for tj in range(8):
            for sx in range(2):
                idx = _cull_rect(pre, tj * 16 + 8 * sx, ti * 16, 8, 16)
                cells.append((ti, tj, sx, idx))
    order = sorted(range(len(cells)), key=lambda i: -len(cells[i][3]))
    groups = [[] for _ in range(N_CORES)]
    loads = [0] * N_CORES
    for i in order:
        g = loads.index(min(loads))
        groups[g].append(i)
        loads[g] += len(cells[i][3]) + 1
    TP = -(-max(loads) // 256) * 256
    if bool(int(os.environ.get("GR_DEBUG", "0"))):
        print(f"[gr] loads={loads} TP={TP}")

    basisT = _make_basisT()
    ident = np.eye(KB, dtype=fp16)
    CH = TP // KB
    SEG0 = 128

    in_maps = []
    core_cells = []
    for core in range(N_CORES):
        cl = [cells[i] for i in groups[core]]
        core_cells.append(cl)
        coef, d, rst = _build_core_arrays(pre, cl, TP, bg)
        # chunk-major d: dxp[p, c*NC3 + j] = d[c*128 + p, j]
        dxp = np.ascontiguousarray(
            d.reshape(CH, KB, NC3).transpose(1, 0, 2).reshape(KB, CH * NC3))
        im = dict(
            bc0=np.concatenate([basisT, coef[:, 0:SEG0]], axis=1),
            ors=np.concatenate([np.ones(KB, np.float32), rst])[None, :],
            dxp=dxp.astype(fp16),
            idn=ident)
        if TP > SEG0:
            im["cfr"] = coef[:, SEG0:]
        in_maps.append(im)

    if TP not in _compiled_cache:
        _compiled_cache[TP] = _build_program(TP)
    nc = _compiled_cache[TP]

    from concourse.bass_utils import run_bass_kernel_spmd
    trace = bool(int(os.environ.get("GR_TRACE", "0")))
    res = run_bass_kernel_spmd(nc, in_maps, list(range(N_CORES)), trace=trace)
    if trace:
        kernel.last_exec_time_ns = res.exec_time_ns
        kernel.last_profile = res.profile_json

    # ---- host scatter ----
    out = np.zeros((3, H, W), np.float32)
    for core in range(N_CORES):
        orgb = np.asarray(res.results[core]["orgb"], np.float32)
        for bi, (ti, tj, sx, idx) in enumerate(core_cells[core]):
            xlo, ylo = tj * 16 + 8 * sx, ti * 16
            blk = orgb[:, 3 * bi:3 * bi + 3].T.reshape(3, 16, 8)
            out[:, ylo:ylo + 16, xlo:xlo + 8] = blk
    return out


# revision 14
# speedup vs baseline: 1.1809x; 1.1809x over previous
"""Trainium2 Bass kernel for a 3D-gaussian-splatting rasterizer.

Transposed ("pixels on partitions") design:
  host (numpy, O(N) work): quaternion -> cov3D -> EWA cov2D -> conic,
    projection, depth sort, per-half-cell (8x16 = 128 px) culling. The 128
    half-cells are LPT-balanced over 8 cores; each core gets one column
    strip: [sep][cell0 gaussians][sep][cell1 gaussians]... padded to TP.
  device (8 NeuronCores, SPMD), partitions = the 128 pixels of whichever
    cell a column belongs to, free dim = gaussian columns:
      P      = basisT^T @ coef                   (one [6,128] stationary)
      alpha  = exp(P)                            (ONE scalar-engine pass)
      om     = 1 - alpha                         (DVE tensor_scalar)
      ts     = scan: state = max(om*state, rst)  (DVE tensor_tensor_scan;
               rst=1 at separator columns resets the running transmittance
               product exactly to 1 between cells -- state is always <=1)
      tt_c   = transpose(ts chunk)               (PE transpose via identity)
      R     += tt_c^T @ d_chunk                  (accumulating rgb matmul)
    where d is the Abel-summation difference array: for cell block b,
      d[sep_b]  = first color (or bg if empty),  d[t] = c_{t+1} - c_t,
      d[last_b] = bg - c_last
    so  rgb_b = sum_t ts[p,t] * d[t, 3b:3b+3]  == front-to-back compositing
    including the residual-transmittance * bg term. No per-cell row caps,
    no chunk chaining, no slot gating, and only ONE exp pass per element.
  host: scatter the [128 px, 3] per-cell blocks into the [3,128,128] image.

Numerically validated (sim2.py): f32 coef/basis + fp16 scan/colors
gives rel err ~6e-3 vs the 2e-2 gate (bf16 coef fails: cancellation).
"""

import os
import numpy as np

N_CORES = 8
H = W = 128
NEG_BIG = -1.0e9
ZNEAR = 0.2
TANFOV = 0.5
FOCAL = W / (2.0 * TANFOV)   # 128.0
KB = 128
NCELL = 16                   # half-cells per core (128/8)
NC3 = 3 * NCELL

_compiled_cache = {}


# ----------------------------------------------------------------------------
# Host-side per-gaussian preprocessing (numpy, O(N))
# ----------------------------------------------------------------------------

def _preprocess(means3D, opacities, colors_precomp, scales, rotations, viewmatrix):
    q = rotations / np.linalg.norm(rotations, axis=-1, keepdims=True)
    r, x, y, z = q[:, 0], q[:, 1], q[:, 2], q[:, 3]
    R = np.stack([
        1 - 2 * (y * y + z * z), 2 * (x * y - r * z), 2 * (x * z + r * y),
        2 * (x * y + r * z), 1 - 2 * (x * x + z * z), 2 * (y * z - r * x),
        2 * (x * z - r * y), 2 * (y * z + r * x), 1 - 2 * (x * x + y * y),
    ], axis=-1).reshape(-1, 3, 3)
    M = R * scales[:, None, :]
    cov3D = np.einsum('nij,nkj->nik', M, M)

    Wm = viewmatrix[:3, :3]
    t = means3D @ Wm.T + viewmatrix[:3, 3]
    tz = t[:, 2]
    lim = 1.3 * TANFOV
    txz = np.clip(t[:, 0] / tz, -lim, lim) * tz
    tyz = np.clip(t[:, 1] / tz, -lim, lim) * tz
    zero = np.zeros_like(tz)
    fx = fy = FOCAL
    J = np.stack([
        np.stack([fx / tz, zero, -fx * txz / (tz * tz)], axis=-1),
        np.stack([zero, fy / tz, -fy * tyz / (tz * tz)], axis=-1),
    ], axis=1)
    T = np.einsum('nij,jk->nik', J, Wm)
    cov2D = np.einsum('nij,njk,nlk->nil', T, cov3D, T)
    a = cov2D[:, 0, 0] + 0.3
    b = cov2D[:, 0, 1]
    c = cov2D[:, 1, 1] + 0.3
    det = a * c - b * b
    det_safe = np.where(det > 0, det, 1.0)
    conA, conB, conC = c / det_safe, -b / det_safe, a / det_safe
    px = fx * t[:, 0] / tz + (W - 1) * 0.5
    py = fy * t[:, 1] / tz + (H - 1) * 0.5
    valid = (det > 0) & (tz > ZNEAR)
    opac = opacities[:, 0]

    # bounding half-widths of the {alpha >= 1/255} ellipse
    ell = np.log(np.maximum(opac * 255.0, 1.0 + 1e-7))
    rx = np.where(valid, np.sqrt(np.maximum(2 * ell * a, 0.0)), 0.0)
    ry = np.where(valid, np.sqrt(np.maximum(2 * ell * c, 0.0)), 0.0)

    order = np.argsort(tz, kind='stable')
    d = dict(conA=conA, conB=conB, conC=conC, px=px, py=py, opac=opac,
             cols=colors_precomp, valid=valid, rx=rx, ry=ry, ell=ell)
    return {k: v[order] for k, v in d.items()}


def _cull_rect(pre, xlo, ylo, w, h):
    """Indices (depth-ordered) of gaussians touching rect, ellipse-corner
    refined."""
    px, py, rx, ry = pre['px'], pre['py'], pre['rx'], pre['ry']
    xhi, yhi = xlo + w - 1, ylo + h - 1
    hit = pre['valid'] & (px + rx >= xlo) & (px - rx <= xhi) \
        & (py + ry >= ylo) & (py - ry <= yhi)
    cx = np.clip(px, xlo, xhi)
    cy = np.clip(py, ylo, yhi)
    dx = cx - px
    dy = cy - py
    beyond = (dx != 0) & (dy != 0)
    quad = pre['conA'] * dx * dx + 2 * pre['conB'] * dx * dy \
        + pre['conC'] * dy * dy
    hit &= ~beyond | (quad <= 2 * pre['ell'])
    return np.nonzero(hit)[0]


def _make_basisT():
    """[6, 128]: rows x^2,y^2,xy,x,y,1 over the 8x16 cell (p = yy*8+xx)."""
    b = np.zeros((6, 128), np.float32)
    for yy in range(16):
        for xx in range(8):
            p = yy * 8 + xx
            xr, yr = xx - 3.5, yy - 7.5
            b[:, p] = [xr * xr, yr * yr, xr * yr, xr, yr, 1.0]
    return b


def _build_core_arrays(pre, cells, TP, bg):
    """coef [6, TP] f32, d [TP, NC3] f32, rst [TP] f32 for one core."""
    coef = np.zeros((6, TP), np.float32)
    coef[5, :] = NEG_BIG
    d = np.zeros((TP, NC3), np.float32)
    rst = np.zeros(TP, np.float32)
    col = 0
    for bi, (ti, tj, sx, idx) in enumerate(cells):
        vx = tj * 16 + 8 * sx + 3.5
        vy = ti * 16 + 7.5
        rst[col] = 1.0
        n = len(idx)
        cols_g = pre['cols'][idx]
        d[col, 3 * bi:3 * bi + 3] = cols_g[0] if n else bg
        col += 1
        if n:
            A, Bc, C = pre['conA'][idx], pre['conB'][idx], pre['conC'][idx]
            pxr = pre['px'][idx] - vx
            pyr = pre['py'][idx] - vy
            sl = slice(col, col + n)
            coef[0, sl] = -0.5 * A
            coef[1, sl] = -0.5 * C
            coef[2, sl] = -Bc
            coef[3, sl] = A * pxr + Bc * pyr
            coef[4, sl] = C * pyr + Bc * pxr
            coef[5, sl] = -0.5 * (A * pxr * pxr + C * pyr * pyr) \
                - Bc * pxr * pyr + np.log(pre['opac'][idx])
            dd = np.empty((n, 3), np.float32)
            dd[:-1] = cols_g[1:] - cols_g[:-1]
            dd[-1] = bg - cols_g[-1]
            d[col:col + n, 3 * bi:3 * bi + 3] = dd
            col += n
    return coef, d, rst


# ----------------------------------------------------------------------------
# Device program
# ----------------------------------------------------------------------------

def _build_program(TP):
    from contextlib import ExitStack
    import concourse.bass as bass  # noqa: F401
    import concourse.tile as tile
    from concourse import mybir, bacc

    f32 = mybir.dt.float32
    f32r = mybir.dt.float32r
    fp16 = mybir.dt.float16
    AF = mybir.ActivationFunctionType
    ALU = mybir.AluOpType

    CH = TP // KB                      # transpose/rgb chunks
    SEG0 = min(512, TP)                # first power/exp/om/scan segment
    segs = [(0, SEG0)] + ([(SEG0, TP)] if TP > SEG0 else [])

    class _BaccOneActSet(bacc.Bacc):
        # Pin Exp to one table set so the scalar engine loads tables once.
        def insert_act_table_loads(self):
            from concourse.hw_specs import get_activation_tables
            from concourse.bacc import _bass_rust
            tables = []
            for name, fns in get_activation_tables(self.m.arch).items():
                if name != 'natural_log_exp_and_others':
                    fns = fns - {AF.Exp}
                tables.append((name, fns))
            _bass_rust.insert_act_table_loads(self, tables)

    nc = _BaccOneActSet(None)
    bc0_d = nc.declare_dram_parameter("bc0", [6, KB + SEG0], f32r,
                                      isOutput=False)
    cfr_d = None
    if TP > SEG0:
        cfr_d = nc.declare_dram_parameter("cfr", [6, TP - SEG0], f32r,
                                          isOutput=False)
    assert segs[0] == (0, SEG0)
    # ors = [ones(128) ++ reset-row(TP)]: the ones give a K=1 broadcast
    # matmul that replicates the reset row across all 128 partitions.
    ors_d = nc.declare_dram_parameter("ors", [1, KB + TP], f32r,
                                      isOutput=False)
    dxp_d = nc.declare_dram_parameter("dxp", [KB, CH * NC3], fp16,
                                      isOutput=False)
    idn_d = nc.declare_dram_parameter("idn", [KB, KB], fp16, isOutput=False)
    orgb_d = nc.declare_dram_parameter("orgb", [KB, NC3], fp16, isOutput=True)

    with ExitStack() as ctx:
        tc = ctx.enter_context(tile.TileContext(
            nc, linearize=bool(int(os.environ.get("GR_LINEARIZE", "0")))))
        const_pool = ctx.enter_context(tc.tile_pool(name="const", bufs=1))
        ps = ctx.enter_context(tc.tile_pool(name="psum", bufs=1, space="PSUM"))

        bc_sb = const_pool.tile([6, KB + TP], f32r)
        ors_sb = const_pool.tile([1, KB + TP], f32r)
        dxp_sb = const_pool.tile([KB, CH * NC3], fp16)
        idn_sb = const_pool.tile([KB, KB], fp16)
        alpha_sb = const_pool.tile([KB, TP], f32)
        om_sb = const_pool.tile([KB, TP], fp16)
        ts_sb = const_pool.tile([KB, TP], fp16)
        tt_sb = const_pool.tile([KB, TP], fp16)
        out_sb = const_pool.tile([KB, NC3], fp16)

        # input DMAs: everything on the sync HW-DGE queue except the big
        # dxp (gpsimd's own queue); the scalar queue stays free so its
        # activation-table loads finish before the first exp needs them.
        nc.sync.dma_start(bc_sb[:, 0:KB + SEG0], bc0_d[:])
        nc.sync.dma_start(ors_sb[:], ors_d[:])
        if cfr_d is not None:
            nc.sync.dma_start(bc_sb[:, KB + SEG0:], cfr_d[:])
        nc.sync.dma_start(idn_sb[:], idn_d[:])
        nc.gpsimd.dma_start(dxp_sb[:], dxp_d[:])

        basisT = bc_sb[:, 0:KB]
        # reset row broadcast to 128 partitions via K=1 matmuls
        r_ps = ps.tile([KB, TP], f32, tag="rps", bufs=1)
        rsegs = [(a, min(a + 512, TP)) for a in range(0, TP, 512)]

        def e_rones(i):
            a, b = rsegs[i]
            nc.tensor.matmul(r_ps[:, a:b], lhsT=ors_sb[:, 0:KB],
                             rhs=ors_sb[:, KB + a:KB + b],
                             start=True, stop=True)

        P = [None] * len(segs)

        def e_pow(si):
            a, b = segs[si]
            P[si] = ps.tile([KB, b - a], f32, tag=f"p{si}", bufs=1,
                            name=f"P{si}")
            nc.tensor.matmul(P[si][:], lhsT=basisT,
                             rhs=bc_sb[:, KB + a:KB + b],
                             start=True, stop=True)

        def e_act(si):
            a, b = segs[si]
            nc.scalar.activation(alpha_sb[:, a:b], P[si][:], AF.Exp)
            nc.vector.tensor_scalar(om_sb[:, a:b], alpha_sb[:, a:b],
                                    -1.0, 1.0, ALU.mult, ALU.add)

        e_pow(0)
        e_rones(0)
        e_act(0)
        for si in range(1, len(segs)):
            e_pow(si)
            if si < len(rsegs):
                e_rones(si)
            e_act(si)
        for i in range(len(segs), len(rsegs)):
            e_rones(i)

        for c in range(CH):
            sl = slice(c * KB, (c + 1) * KB)
            nc.vector.tensor_tensor_scan(
                ts_sb[:, sl], om_sb[:, sl], r_ps[:, sl],
                initial=(1.0 if c == 0 else ts_sb[:, c * KB - 1:c * KB]),
                op0=ALU.mult, op1=ALU.max)

        R = ps.tile([KB, NC3], f32, tag="r", bufs=1)
        for c in range(CH):
            sl = slice(c * KB, (c + 1) * KB)
            TPc = ps.tile([KB, KB], fp16, tag="tp", bufs=2, name=f"TP{c}")
            nc.tensor.transpose(TPc[:], ts_sb[:, sl], idn_sb[:])
            # PSUM -> SBUF chunk copies alternate scalar/vector engines
            # (gpsimd cannot access PSUM)
            if c % 2 == 0:
                nc.scalar.copy(tt_sb[:, sl], TPc[:])
            else:
                nc.vector.tensor_copy(tt_sb[:, sl], TPc[:])
            nc.tensor.matmul(R[:], lhsT=tt_sb[:, sl],
                             rhs=dxp_sb[:, c * NC3:(c + 1) * NC3],
                             start=(c == 0), stop=(c == CH - 1))

        nc.vector.tensor_copy(out_sb[:], R[:])
        nc.sync.dma_start(orgb_d[:], out_sb[:])

    nc.compile()
    return nc


# ----------------------------------------------------------------------------
# Entry point
# ----------------------------------------------------------------------------

def kernel(means3D, means2D, opacities, colors_precomp, scales, rotations,
           bg, viewmatrix):
    import ml_dtypes
    fp16 = np.float16
    means3D = np.asarray(means3D, np.float32)
    opacities = np.asarray(opacities, np.float32)
    colors_precomp = np.asarray(colors_precomp, np.float32)
    scales = np.asarray(scales, np.float32)
    rotations = np.asarray(rotations, np.float32)
    bg = np.asarray(bg, np.float32)
    viewmatrix = np.asarray(viewmatrix, np.float32)

    pre = _preprocess(means3D, opacities, colors_precomp, scales, rotations,
                      viewmatrix)

    # per half-cell culled lists, LPT-balanced over cores
    cells = []
    for ti in range(8):
        for tj in range(8):
            for sx in range(2):
                idx = _cull_rect(pre, tj * 16 + 8 * sx, ti * 16, 8, 16)
                cells.append((ti, tj, sx, idx))
    order = sorted(range(len(cells)), key=lambda i: -len(cells[i][3]))
    groups = [[] for _ in range(N_CORES)]
    loads = [0] * N_CORES
    for i in order:
        g = loads.index(min(loads))
        groups[g].append(i)
        loads[g] += len(cells[i][3]) + 1
    TP = -(-max(loads) // 256) * 256
    if bool(int(os.environ.get("GR_DEBUG", "0"))):
        print(f"[gr] loads={loads} TP={TP}")

    basisT = _make_basisT()
    ident = np.eye(KB, dtype=fp16)
    CH = TP // KB
    SEG0 = min(512, TP)

    in_maps = []
    core_cells = []
    for core in range(N_CORES):
        cl = [cells[i] for i in groups[core]]
        core_cells.append(cl)
        coef, d, rst = _build_core_arrays(pre, cl, TP, bg)
        # chunk-major d: dxp[p, c*NC3 + j] = d[c*128 + p, j]
        dxp = np.ascontiguousarray(
            d.reshape(CH, KB, NC3).transpose(1, 0, 2).reshape(KB, CH * NC3))
        im = dict(
            bc0=np.concatenate([basisT, coef[:, 0:SEG0]], axis=1),
            ors=np.concatenate([np.ones(KB, np.float32), rst])[None, :],
            dxp=dxp.astype(fp16),
            idn=ident)
        if TP > SEG0:
            im["cfr"] = coef[:, SEG0:]
        in_maps.append(im)

    if TP not in _compiled_cache:
        _compiled_cache[TP] = _build_program(TP)
    nc = _compiled_cache[TP]

    from concourse.bass_utils import run_bass_kernel_spmd
    trace = bool(int(os.environ.get("GR_TRACE", "0")))
    res = run_bass_kernel_spmd(nc, in_maps, list(range(N_CORES)), trace=trace)
    if trace:
        kernel.last_exec_time_ns = res.exec_time_ns
        kernel.last_profile = res.profile_json

    # ---- host scatter ----
    out = np.zeros((3, H, W), np.float32)
    for core in range(N_CORES):
        orgb = np.asarray(res.results[core]["orgb"], np.float32)
        for bi, (ti, tj, sx, idx) in enumerate(core_cells[core]):
            xlo, ylo = tj * 16 + 8 * sx, ti * 16
            blk = orgb[:, 3 * bi:3 * bi + 3].T.reshape(3, 16, 8)
            out[:, ylo:ylo + 16, xlo:xlo + 8] = blk
    return out


# revision 15
# speedup vs baseline: 1.2055x; 1.0209x over previous
"""Trainium2 Bass kernel for a 3D-gaussian-splatting rasterizer.

Transposed ("pixels on partitions") design:
  host (numpy, O(N) work): quaternion -> cov3D -> EWA cov2D -> conic,
    projection, depth sort, per-half-cell (8x16 = 128 px) culling. The 128
    half-cells are LPT-balanced over 8 cores; each core gets one column
    strip: [sep][cell0 gaussians][sep][cell1 gaussians]... padded to TP.
  device (8 NeuronCores, SPMD), partitions = the 128 pixels of whichever
    cell a column belongs to, free dim = gaussian columns:
      P      = basisT^T @ coef                   (one [6,128] stationary)
      alpha  = exp(P)                            (ONE scalar-engine pass)
      om     = 1 - alpha                         (DVE tensor_scalar)
      ts     = scan: state = max(om*state, rst)  (DVE tensor_tensor_scan;
               rst=1 at separator columns resets the running transmittance
               product exactly to 1 between cells -- state is always <=1)
      tt_c   = transpose(ts chunk)               (PE transpose via identity)
      R     += tt_c^T @ d_chunk                  (accumulating rgb matmul)
    where d is the Abel-summation difference array: for cell block b,
      d[sep_b]  = first color (or bg if empty),  d[t] = c_{t+1} - c_t,
      d[last_b] = bg - c_last
    so  rgb_b = sum_t ts[p,t] * d[t, 3b:3b+3]  == front-to-back compositing
    including the residual-transmittance * bg term. No per-cell row caps,
    no chunk chaining, no slot gating, and only ONE exp pass per element.
  host: scatter the [128 px, 3] per-cell blocks into the [3,128,128] image.

Numerically validated (sim2.py): f32 coef/basis + fp16 scan/colors
gives rel err ~6e-3 vs the 2e-2 gate (bf16 coef fails: cancellation).
"""

import os
import numpy as np

N_CORES = 8
H = W = 128
NEG_BIG = -1.0e9
ZNEAR = 0.2
TANFOV = 0.5
FOCAL = W / (2.0 * TANFOV)   # 128.0
KB = 128
NCELL = 16                   # half-cells per core (128/8)
NC3 = 3 * NCELL

_compiled_cache = {}


# ----------------------------------------------------------------------------
# Host-side per-gaussian preprocessing (numpy, O(N))
# ----------------------------------------------------------------------------

def _preprocess(means3D, opacities, colors_precomp, scales, rotations, viewmatrix):
    q = rotations / np.linalg.norm(rotations, axis=-1, keepdims=True)
    r, x, y, z = q[:, 0], q[:, 1], q[:, 2], q[:, 3]
    R = np.stack([
        1 - 2 * (y * y + z * z), 2 * (x * y - r * z), 2 * (x * z + r * y),
        2 * (x * y + r * z), 1 - 2 * (x * x + z * z), 2 * (y * z - r * x),
        2 * (x * z - r * y), 2 * (y * z + r * x), 1 - 2 * (x * x + y * y),
    ], axis=-1).reshape(-1, 3, 3)
    M = R * scales[:, None, :]
    cov3D = np.einsum('nij,nkj->nik', M, M)

    Wm = viewmatrix[:3, :3]
    t = means3D @ Wm.T + viewmatrix[:3, 3]
    tz = t[:, 2]
    lim = 1.3 * TANFOV
    txz = np.clip(t[:, 0] / tz, -lim, lim) * tz
    tyz = np.clip(t[:, 1] / tz, -lim, lim) * tz
    zero = np.zeros_like(tz)
    fx = fy = FOCAL
    J = np.stack([
        np.stack([fx / tz, zero, -fx * txz / (tz * tz)], axis=-1),
        np.stack([zero, fy / tz, -fy * tyz / (tz * tz)], axis=-1),
    ], axis=1)
    T = np.einsum('nij,jk->nik', J, Wm)
    cov2D = np.einsum('nij,njk,nlk->nil', T, cov3D, T)
    a = cov2D[:, 0, 0] + 0.3
    b = cov2D[:, 0, 1]
    c = cov2D[:, 1, 1] + 0.3
    det = a * c - b * b
    det_safe = np.where(det > 0, det, 1.0)
    conA, conB, conC = c / det_safe, -b / det_safe, a / det_safe
    px = fx * t[:, 0] / tz + (W - 1) * 0.5
    py = fy * t[:, 1] / tz + (H - 1) * 0.5
    valid = (det > 0) & (tz > ZNEAR)
    opac = opacities[:, 0]

    # bounding half-widths of the {alpha >= 1/255} ellipse
    ell = np.log(np.maximum(opac * 255.0, 1.0 + 1e-7))
    rx = np.where(valid, np.sqrt(np.maximum(2 * ell * a, 0.0)), 0.0)
    ry = np.where(valid, np.sqrt(np.maximum(2 * ell * c, 0.0)), 0.0)

    order = np.argsort(tz, kind='stable')
    d = dict(conA=conA, conB=conB, conC=conC, px=px, py=py, opac=opac,
             cols=colors_precomp, valid=valid, rx=rx, ry=ry, ell=ell)
    return {k: v[order] for k, v in d.items()}


def _cull_rect(pre, xlo, ylo, w, h):
    """Indices (depth-ordered) of gaussians touching rect, ellipse-corner
    refined."""
    px, py, rx, ry = pre['px'], pre['py'], pre['rx'], pre['ry']
    xhi, yhi = xlo + w - 1, ylo + h - 1
    hit = pre['valid'] & (px + rx >= xlo) & (px - rx <= xhi) \
        & (py + ry >= ylo) & (py - ry <= yhi)
    cx = np.clip(px, xlo, xhi)
    cy = np.clip(py, ylo, yhi)
    dx = cx - px
    dy = cy - py
    beyond = (dx != 0) & (dy != 0)
    quad = pre['conA'] * dx * dx + 2 * pre['conB'] * dx * dy \
        + pre['conC'] * dy * dy
    hit &= ~beyond | (quad <= 2 * pre['ell'])
    return np.nonzero(hit)[0]


def _make_basisT():
    """[6, 128]: rows x^2,y^2,xy,x,y,1 over the 8x16 cell (p = yy*8+xx)."""
    b = np.zeros((6, 128), np.float32)
    for yy in range(16):
        for xx in range(8):
            p = yy * 8 + xx
            xr, yr = xx - 3.5, yy - 7.5
            b[:, p] = [xr * xr, yr * yr, xr * yr, xr, yr, 1.0]
    return b


def _build_core_arrays(pre, cells, TP, bg):
    """coef [6, TP] f32, d [TP, NC3] f32, rst [TP] f32 for one core."""
    coef = np.zeros((6, TP), np.float32)
    coef[5, :] = NEG_BIG
    d = np.zeros((TP, NC3), np.float32)
    rst = np.zeros(TP, np.float32)
    col = 0
    for bi, (ti, tj, sx, idx) in enumerate(cells):
        vx = tj * 16 + 8 * sx + 3.5
        vy = ti * 16 + 7.5
        rst[col] = 1.0
        n = len(idx)
        cols_g = pre['cols'][idx]
        d[col, 3 * bi:3 * bi + 3] = cols_g[0] if n else bg
        col += 1
        if n:
            A, Bc, C = pre['conA'][idx], pre['conB'][idx], pre['conC'][idx]
            pxr = pre['px'][idx] - vx
            pyr = pre['py'][idx] - vy
            sl = slice(col, col + n)
            coef[0, sl] = -0.5 * A
            coef[1, sl] = -0.5 * C
            coef[2, sl] = -Bc
            coef[3, sl] = A * pxr + Bc * pyr
            coef[4, sl] = C * pyr + Bc * pxr
            coef[5, sl] = -0.5 * (A * pxr * pxr + C * pyr * pyr) \
                - Bc * pxr * pyr + np.log(pre['opac'][idx])
            dd = np.empty((n, 3), np.float32)
            dd[:-1] = cols_g[1:] - cols_g[:-1]
            dd[-1] = bg - cols_g[-1]
            d[col:col + n, 3 * bi:3 * bi + 3] = dd
            col += n
    return coef, d, rst


# ----------------------------------------------------------------------------
# Device program
# ----------------------------------------------------------------------------

def _build_program(TP):
    from contextlib import ExitStack
    import concourse.bass as bass  # noqa: F401
    import concourse.tile as tile
    from concourse import mybir, bacc

    f32 = mybir.dt.float32
    f32r = mybir.dt.float32r
    fp16 = mybir.dt.float16
    AF = mybir.ActivationFunctionType
    ALU = mybir.AluOpType

    CH = TP // KB                      # transpose/rgb chunks
    # small first segment so the first scan chunk starts early
    if TP > 512:
        segs = [(0, 128), (128, 512), (512, TP)]
    elif TP > 128:
        segs = [(0, 128), (128, TP)]
    else:
        segs = [(0, TP)]
    SEG0 = 128

    class _BaccOneActSet(bacc.Bacc):
        # Pin Exp to one table set so the scalar engine loads tables once.
        def insert_act_table_loads(self):
            from concourse.hw_specs import get_activation_tables
            from concourse.bacc import _bass_rust
            tables = []
            for name, fns in get_activation_tables(self.m.arch).items():
                if name != 'natural_log_exp_and_others':
                    fns = fns - {AF.Exp}
                tables.append((name, fns))
            _bass_rust.insert_act_table_loads(self, tables)

    nc = _BaccOneActSet(None)
    bc0_d = nc.declare_dram_parameter("bc0", [6, KB + SEG0], f32r,
                                      isOutput=False)
    cfr_d = None
    if TP > SEG0:
        cfr_d = nc.declare_dram_parameter("cfr", [6, TP - SEG0], f32r,
                                          isOutput=False)
    assert segs[0] == (0, SEG0)
    # ors = [ones(128) ++ reset-row(TP)]: the ones give a K=1 broadcast
    # matmul that replicates the reset row across all 128 partitions.
    ors_d = nc.declare_dram_parameter("ors", [1, KB + TP], f32r,
                                      isOutput=False)
    dxp_d = nc.declare_dram_parameter("dxp", [KB, CH * NC3], fp16,
                                      isOutput=False)
    idn_d = nc.declare_dram_parameter("idn", [KB, KB], fp16, isOutput=False)
    orgb_d = nc.declare_dram_parameter("orgb", [KB, NC3], fp16, isOutput=True)

    with ExitStack() as ctx:
        tc = ctx.enter_context(tile.TileContext(
            nc, linearize=bool(int(os.environ.get("GR_LINEARIZE", "0")))))
        const_pool = ctx.enter_context(tc.tile_pool(name="const", bufs=1))
        ps = ctx.enter_context(tc.tile_pool(name="psum", bufs=1, space="PSUM"))

        bc_sb = const_pool.tile([6, KB + TP], f32r)
        ors_sb = const_pool.tile([1, KB + TP], f32r)
        dxp_sb = const_pool.tile([KB, CH * NC3], fp16)
        idn_sb = const_pool.tile([KB, KB], fp16)
        alpha_sb = const_pool.tile([KB, TP], f32)
        om_sb = const_pool.tile([KB, TP], fp16)
        ts_sb = const_pool.tile([KB, TP], fp16)
        tt_sb = const_pool.tile([KB, TP], fp16)
        out_sb = const_pool.tile([KB, NC3], fp16)

        # input DMAs: everything on the sync HW-DGE queue except the big
        # dxp (gpsimd's own queue); the scalar queue stays free so its
        # activation-table loads finish before the first exp needs them.
        nc.sync.dma_start(bc_sb[:, 0:KB + SEG0], bc0_d[:])
        nc.sync.dma_start(ors_sb[:], ors_d[:])
        if cfr_d is not None:
            nc.sync.dma_start(bc_sb[:, KB + SEG0:], cfr_d[:])
        nc.sync.dma_start(idn_sb[:], idn_d[:])
        nc.gpsimd.dma_start(dxp_sb[:], dxp_d[:])

        basisT = bc_sb[:, 0:KB]
        # reset row broadcast to 128 partitions via K=1 matmuls
        r_ps = ps.tile([KB, TP], f32, tag="rps", bufs=1)
        rsegs = [(a, min(a + 512, TP)) for a in range(0, TP, 512)]

        def e_rones(i):
            a, b = rsegs[i]
            nc.tensor.matmul(r_ps[:, a:b], lhsT=ors_sb[:, 0:KB],
                             rhs=ors_sb[:, KB + a:KB + b],
                             start=True, stop=True)

        P = [None] * len(segs)

        def e_pow(si):
            a, b = segs[si]
            P[si] = ps.tile([KB, b - a], f32, tag=f"p{si}", bufs=1,
                            name=f"P{si}")
            nc.tensor.matmul(P[si][:], lhsT=basisT,
                             rhs=bc_sb[:, KB + a:KB + b],
                             start=True, stop=True)

        def e_act(si):
            a, b = segs[si]
            nc.scalar.activation(alpha_sb[:, a:b], P[si][:], AF.Exp)
            nc.vector.tensor_scalar(om_sb[:, a:b], alpha_sb[:, a:b],
                                    -1.0, 1.0, ALU.mult, ALU.add)

        e_pow(0)
        e_rones(0)
        e_act(0)
        for si in range(1, len(segs)):
            e_pow(si)
            if si < len(rsegs):
                e_rones(si)
            e_act(si)
        for i in range(len(segs), len(rsegs)):
            e_rones(i)

        for c in range(CH):
            sl = slice(c * KB, (c + 1) * KB)
            nc.vector.tensor_tensor_scan(
                ts_sb[:, sl], om_sb[:, sl], r_ps[:, sl],
                initial=(1.0 if c == 0 else ts_sb[:, c * KB - 1:c * KB]),
                op0=ALU.mult, op1=ALU.max)

        R = ps.tile([KB, NC3], f32, tag="r", bufs=1)
        for c in range(CH):
            sl = slice(c * KB, (c + 1) * KB)
            TPc = ps.tile([KB, KB], fp16, tag="tp", bufs=2, name=f"TP{c}")
            nc.tensor.transpose(TPc[:], ts_sb[:, sl], idn_sb[:])
            # PSUM -> SBUF chunk copies alternate scalar/vector engines
            # (gpsimd cannot access PSUM)
            if c % 2 == 0:
                nc.scalar.copy(tt_sb[:, sl], TPc[:])
            else:
                nc.vector.tensor_copy(tt_sb[:, sl], TPc[:])
            nc.tensor.matmul(R[:], lhsT=tt_sb[:, sl],
                             rhs=dxp_sb[:, c * NC3:(c + 1) * NC3],
                             start=(c == 0), stop=(c == CH - 1))

        nc.vector.tensor_copy(out_sb[:], R[:])
        nc.sync.dma_start(orgb_d[:], out_sb[:])

    nc.compile()
    return nc


# ----------------------------------------------------------------------------
# Entry point
# ----------------------------------------------------------------------------

def kernel(means3D, means2D, opacities, colors_precomp, scales, rotations,
           bg, viewmatrix):
    import ml_dtypes
    fp16 = np.float16
    means3D = np.asarray(means3D, np.float32)
    opacities = np.asarray(opacities, np.float32)
    colors_precomp = np.asarray(colors_precomp, np.float32)
    scales = np.asarray(scales, np.float32)
    rotations = np.asarray(rotations, np.float32)
    bg = np.asarray(bg, np.float32)
    viewmatrix = np.asarray(viewmatrix, np.float32)

    pre = _preprocess(means3D, opacities, colors_precomp, scales, rotations,
                      viewmatrix)

    # per half-cell culled lists, LPT-balanced over cores
    cells = []
    for ti in range(8):
        for tj in range(8):
            for sx in range(2):
                idx = _cull_rect(pre, tj * 16 + 8 * sx, ti * 16, 8, 16)
                cells.append((ti, tj, sx, idx))
    order = sorted(range(len(cells)), key=lambda i: -len(cells[i][3]))
    groups = [[] for _ in range(N_CORES)]
    loads = [0] * N_CORES
    for i in order:
        g = loads.index(min(loads))
        groups[g].append(i)
        loads[g] += len(cells[i][3]) + 1
    TP = -(-max(loads) // 256) * 256
    if bool(int(os.environ.get("GR_DEBUG", "0"))):
        print(f"[gr] loads={loads} TP={TP}")

    basisT = _make_basisT()
    ident = np.eye(KB, dtype=fp16)
    CH = TP // KB
    SEG0 = 128

    in_maps = []
    core_cells = []
    for core in range(N_CORES):
        cl = [cells[i] for i in groups[core]]
        core_cells.append(cl)
        coef, d, rst = _build_core_arrays(pre, cl, TP, bg)
        # chunk-major d: dxp[p, c*NC3 + j] = d[c*128 + p, j]
        dxp = np.ascontiguousarray(
            d.reshape(CH, KB, NC3).transpose(1, 0, 2).reshape(KB, CH * NC3))
        im = dict(
            bc0=np.concatenate([basisT, coef[:, 0:SEG0]], axis=1),
            ors=np.concatenate([np.ones(KB, np.float32), rst])[None, :],
            dxp=dxp.astype(fp16),
            idn=ident)
        if TP > SEG0:
            im["cfr"] = coef[:, SEG0:]
        in_maps.append(im)

    if TP not in _compiled_cache:
        _compiled_cache[TP] = _build_program(TP)
    nc = _compiled_cache[TP]

    from concourse.bass_utils import run_bass_kernel_spmd
    trace = bool(int(os.environ.get("GR_TRACE", "0")))
    res = run_bass_kernel_spmd(nc, in_maps, list(range(N_CORES)), trace=trace)
    if trace:
        kernel.last_exec_time_ns = res.exec_time_ns
        kernel.last_profile = res.profile_json

    # ---- host scatter ----
    out = np.zeros((3, H, W), np.float32)
    for core in range(N_CORES):
        orgb = np.asarray(res.results[core]["orgb"], np.float32)
        for bi, (ti, tj, sx, idx) in enumerate(core_cells[core]):
            xlo, ylo = tj * 16 + 8 * sx, ti * 16
            blk = orgb[:, 3 * bi:3 * bi + 3].T.reshape(3, 16, 8)
            out[:, ylo:ylo + 16, xlo:xlo + 8] = blk
    return out
